# revision 8
# baseline (speedup 1.0000x reference)
"""Trainium2 Bass kernel for nn_Axial_PFCU_DNAS (dense_cnn).

Contract: kernel(**inputs) takes the FULL unsharded inputs (x [2,256,224,224] f32
plus the params pytree), distributes across 8 NeuronCores internally
(data-parallel: 2 batches x 4 H-slabs of 56 rows), and returns the FULL
[2,256,224,224] f32 output.

Self-contained: hardcodes all shapes; only depends on the container's
/opt/trn_rl_repo runtime.
"""

import math
import os
import sys

sys.path.insert(0, "/opt/trn_rl_repo")

import numpy as np

import concourse.bass as bass
import concourse.bacc as bacc
import concourse.mybir as mybir
import concourse.tile as tile
from concourse.bass_utils import run_bass_kernel_spmd

F32 = mybir.dt.float32
AF = mybir.ActivationFunctionType
ALU = mybir.AluOpType

# ---- problem constants -------------------------------------------------------
B, C, H, W = 2, 256, 224, 224
MIP = 8
EPS = 1e-5
NCORES = 8
SH = 56          # output rows per core
HALO = 16        # max |offset| of any tap
HIN = SH + 2 * HALO      # 88 input rows per core slab
WPAD = W + 2 * HALO      # 256 padded width
SS = 8           # stripe height (output rows per stripe)
NS = SH // SS    # 7 stripes
SIN = SS + 2 * HALO      # 40 input rows per stripe
PIX = SH * W     # 12544 pixels per core
SPIX = SS * W    # 1792 pixels per stripe
NCHUNK = 4       # psum chunks per stripe (448 wide)
CW = SPIX // NCHUNK

LAST_RESULTS = None  # set by kernel(); test.py reads exec_time_ns from it


# ---- host-side constant folding ---------------------------------------------
def _np(v):
    return np.asarray(v, dtype=np.float32)


def _bn_fold(p):
    s = _np(p["gamma"]) / np.sqrt(_np(p["var"]) + EPS)
    b = _np(p["beta"]) - _np(p["mean"]) * s
    return s, b


def _tap_coefs(r, w):
    """w: [C, 5] tap weights; returns dict offset -> [C] coef."""
    r = max(float(r), 1.0)
    d = {}
    for i in range(5):
        s = (i - 2) * r
        i0 = math.floor(s)
        f = s - i0
        for dd, wt in ((i0, 1.0 - f), (i0 + 1, f)):
            if wt != 0.0:
                d[dd] = d.get(dd, 0.0) + wt * _np(w)[:, i]
    return d


def _merge(a, b):
    out = dict(a)
    for k, v in b.items():
        out[k] = out.get(k, 0.0) + v
    return out


def fold_constants(params):
    p = params
    cH = _merge(_tap_coefs(p["r_m"], p["wm_h"]), _tap_coefs(p["r_l"], p["wl_h"]))
    cW = _merge(_tap_coefs(p["r_m"], p["wm_w"]), _tap_coefs(p["r_l"], p["wl_w"]))
    cH[0] = cH.get(0, 0.0) + 2.0  # the two residual x terms of (m + l)
    offs_h = sorted(cH)
    offs_w = sorted(cW)
    coefH = np.stack([np.broadcast_to(cH[d], (C,)) for d in offs_h], 1).astype(np.float32)
    coefW = np.stack([np.broadcast_to(cW[d], (C,)) for d in offs_w], 1).astype(np.float32)

    sf, bf = _bn_fold(p["bn_fuse"])
    wf = _np(p["pw_fuse_w"])          # [O, C]
    wfT = (wf * sf[:, None]).T.copy() # [C, O] lhsT with BN scale folded

    sa, sb = _bn_fold(p["dg_bn"])
    wh = _np(p["dg_wh"])[:, 0, :, 0]  # [C, 3] taps at h-1, h, h+1
    ww = _np(p["dg_ww"])[:, 0, 0, :]  # [C, 3] taps at w-1, w, w+1
    anch = np.stack(
        [
            sa * (1.0 + wh[:, 1] + ww[:, 1]),  # center
            sa * wh[:, 0],                     # h-1
            sa * wh[:, 2],                     # h+1
            sa * ww[:, 0],                     # w-1
            sa * ww[:, 2],                     # w+1
        ],
        1,
    ).astype(np.float32)

    bias = (bf + sb).astype(np.float32)
    alpha = _np(p["act_alpha"])

    ca = p["ca"]
    s1, b1 = _bn_fold(ca["bn1"])
    c1T = (_np(ca["conv1_w"]) / 224.0).T.copy()  # [C, MIP], pooling mean folded
    chT = _np(ca["convh_w"]).T.copy()            # [MIP, C]
    cwT = _np(ca["convw_w"]).T.copy()            # [MIP, C]
    alpha1 = _np(ca["act_alpha"])

    return dict(
        offs_h=offs_h, offs_w=offs_w, coefH=coefH, coefW=coefW, wfT=wfT,
        anch=anch, bias=bias, alpha=alpha, c1T=c1T, chT=chT, cwT=cwT,
        s1=s1, b1=b1, alpha1=alpha1,
    )


# ---- bass program ------------------------------------------------------------
_CACHE = {}


def build_program(offs_h, offs_w):
    nc = bacc.Bacc("TRN2", target_bir_lowering=False, debug=False,
                   num_devices=NCORES)

    def inp(name, shape):
        return nc.dram_tensor(name, list(shape), F32, kind="ExternalInput").ap()

    xg = [inp(f"x{g}", (128, HIN * WPAD)) for g in range(2)]
    coefH_t = [inp(f"coefH{g}", (128, len(offs_h))) for g in range(2)]
    coefW_t = [inp(f"coefW{g}", (128, len(offs_w))) for g in range(2)]
    anch_t = [inp(f"anch{g}", (128, 5)) for g in range(2)]
    wfT_t = [inp(f"wfT{g}", (128, 256)) for g in range(2)]
    bias_t = [inp(f"bias{g}", (128, 1)) for g in range(2)]
    alpha_t = [inp(f"palpha{g}", (128, 1)) for g in range(2)]
    c1T_t = [inp(f"c1T{g}", (128, MIP)) for g in range(2)]
    chT_t = inp("chT", (MIP, 256))
    cwT_t = inp("cwT", (MIP, 256))
    s1_t = inp("s1", (MIP, 1))
    b1_t = inp("b1", (MIP, 1))
    alpha1_t = inp("alpha1", (MIP, 1))

    out_t = [
        nc.dram_tensor(f"out{g}", [128, PIX], F32, kind="ExternalOutput").ap()
        for g in range(2)
    ]

    with tile.TileContext(nc) as tc:
        with (
            tc.tile_pool(name="consts", bufs=1) as cpool,
            tc.tile_pool(name="xin", bufs=1) as xpool,
            tc.tile_pool(name="work", bufs=2) as wpool,
            tc.tile_pool(name="acc", bufs=1) as apool,
            tc.tile_pool(name="psum", bufs=4, space="PSUM") as pspool,
            tc.tile_pool(name="psmall", bufs=1, space="PSUM") as pspool2,
            tc.tile_pool(name="dram", bufs=1, space="DRAM") as dpool,
        ):
            # --- load constants once ---
            def cload(ap, shape, tag):
                t = cpool.tile(list(shape), F32, tag=tag, name=tag)
                nc.sync.dma_start(t[:], ap)
                return t

            coefH = [cload(coefH_t[g], (128, len(offs_h)), f"cH{g}") for g in range(2)]
            coefW = [cload(coefW_t[g], (128, len(offs_w)), f"cW{g}") for g in range(2)]
            anch = [cload(anch_t[g], (128, 5), f"an{g}") for g in range(2)]
            wfT = [cload(wfT_t[g], (128, 256), f"wf{g}") for g in range(2)]
            bias = [cload(bias_t[g], (128, 1), f"bi{g}") for g in range(2)]
            alpha = [cload(alpha_t[g], (128, 1), f"al{g}") for g in range(2)]
            c1T = [cload(c1T_t[g], (128, MIP), f"c1{g}") for g in range(2)]
            chT = cload(chT_t, (MIP, 256), "chT")
            cwT = cload(cwT_t, (MIP, 256), "cwT")
            s1 = cload(s1_t, (MIP, 1), "s1")
            b1 = cload(b1_t, (MIP, 1), "b1")
            alpha1 = cload(alpha1_t, (MIP, 1), "alpha1")

            # --- accumulators ---
            xh = [apool.tile([128, SH], F32, tag=f"xh{g}", name=f"xh{g}") for g in range(2)]
            xw = [apool.tile([128, W], F32, tag=f"xw{g}", name=f"xw{g}") for g in range(2)]
            outp_dram = [dpool.tile([128, PIX], F32, tag=f"od{g}", name=f"od{g}") for g in range(2)]

            # --- main stripe loop: stencils + pw conv + prelu + pools ---
            for t in range(NS):
                r0 = t * SS
                xin = [None, None]
                s_t = [None, None]
                a_t = [None, None]
                for g in range(2):
                    xin[g] = xpool.tile([128, SIN * WPAD], F32, tag=f"xin{g}", name=f"xin{g}_{t}")
                    nc.gpsimd.dma_start(
                        xin[g][:], xg[g][:, r0 * WPAD:(r0 + SIN) * WPAD]
                    )
                    x3 = xin[g].rearrange("p (h w) -> p h w", w=WPAD)

                    def xv(dh, dw, x3=x3):
                        return x3[:, HALO + dh:HALO + dh + SS,
                                  HALO + dw:HALO + dw + W]

                    s_t[g] = wpool.tile([128, SPIX], F32, tag=f"s{g}", name=f"s{g}_{t}")
                    s3 = s_t[g].rearrange("p (h w) -> p h w", w=W)
                    nc.vector.tensor_scalar_mul(s3, xv(offs_h[0], 0),
                                                coefH[g][:, 0:1])
                    for i, d in enumerate(offs_h[1:], start=1):
                        nc.vector.scalar_tensor_tensor(
                            s3, xv(d, 0), coefH[g][:, i:i + 1], s3,
                            ALU.mult, ALU.add)
                    for i, d in enumerate(offs_w):
                        nc.vector.scalar_tensor_tensor(
                            s3, xv(0, d), coefW[g][:, i:i + 1], s3,
                            ALU.mult, ALU.add)

                    a_t[g] = wpool.tile([128, SPIX], F32, tag=f"a{g}", name=f"a{g}_{t}")
                    a3 = a_t[g].rearrange("p (h w) -> p h w", w=W)
                    nc.vector.tensor_scalar_mul(a3, xv(0, 0), anch[g][:, 0:1])
                    for i, (dh, dw) in enumerate([(-1, 0), (1, 0), (0, -1), (0, 1)],
                                                 start=1):
                        nc.vector.scalar_tensor_tensor(
                            a3, xv(dh, dw), anch[g][:, i:i + 1], a3,
                            ALU.mult, ALU.add)

                for og in range(2):
                    outp = wpool.tile([128, SPIX], F32, tag=f"outp{og}")
                    for ck in range(NCHUNK):
                        sl = slice(ck * CW, (ck + 1) * CW)
                        ps = pspool.tile([128, CW], F32, tag="pw")
                        nc.tensor.matmul(ps[:], wfT[0][:, og * 128:(og + 1) * 128],
                                         s_t[0][:, sl], start=True, stop=False)
                        nc.tensor.matmul(ps[:], wfT[1][:, og * 128:(og + 1) * 128],
                                         s_t[1][:, sl], start=False, stop=True)
                        tt = wpool.tile([128, CW], F32, tag="tt")
                        nc.vector.scalar_tensor_tensor(
                            tt[:], ps[:], 1.0, a_t[og][:, sl], ALU.mult, ALU.add)
                        nc.scalar.activation(outp[:, sl], tt[:], AF.Prelu,
                                             bias=bias[og][:, 0:1], scale=1.0,
                                             alpha=alpha[og][:, 0:1])
                    o3 = outp.rearrange("p (h w) -> p h w", w=W)
                    nc.vector.tensor_reduce(xh[og][:, r0:r0 + SS], o3,
                                            axis=mybir.AxisListType.X, op=ALU.add)
                    o3t = outp.rearrange("p (h w) -> p w h", w=W)
                    if t == 0:
                        nc.vector.tensor_reduce(xw[og][:], o3t,
                                                axis=mybir.AxisListType.X,
                                                op=ALU.add)
                    else:
                        rw = wpool.tile([128, W], F32, tag="rw")
                        nc.vector.tensor_reduce(rw[:], o3t,
                                                axis=mybir.AxisListType.X,
                                                op=ALU.add)
                        nc.vector.tensor_add(xw[og][:], xw[og][:], rw[:])
                    nc.sync.dma_start(outp_dram[og][:, r0 * W:(r0 + SS) * W],
                                      outp[:])

            # --- all-reduce the H-pool partial sums across the 4 slab cores ---
            cc_in = dpool.tile([256, W], F32, tag="ccin")
            cc_out = dpool.tile([256, W], F32, tag="ccout")
            for og in range(2):
                nc.sync.dma_start(cc_in[og * 128:(og + 1) * 128, :], xw[og][:])
            nc.gpsimd.collective_compute(
                "AllReduce", ALU.add,
                replica_groups=[[0, 1, 2, 3], [4, 5, 6, 7]],
                ins=[cc_in.opt()], outs=[cc_out.opt()],
            )
            xwsum = [apool.tile([128, W], F32, tag=f"xws{g}", name=f"xws{g}") for g in range(2)]
            for og in range(2):
                nc.sync.dma_start(xwsum[og][:], cc_out[og * 128:(og + 1) * 128, :])

            # --- CoordAtt bottleneck ---
            ps_y = pspool2.tile([MIP, SH + W], F32, tag="psy")
            nc.tensor.matmul(ps_y[:, 0:SH], c1T[0][:], xh[0][:],
                             start=True, stop=False)
            nc.tensor.matmul(ps_y[:, 0:SH], c1T[1][:], xh[1][:],
                             start=False, stop=True)
            nc.tensor.matmul(ps_y[:, SH:], c1T[0][:], xwsum[0][:],
                             start=True, stop=False)
            nc.tensor.matmul(ps_y[:, SH:], c1T[1][:], xwsum[1][:],
                             start=False, stop=True)
            y = apool.tile([MIP, SH + W], F32, tag="y")
            nc.scalar.activation(y[:], ps_y[:], AF.Prelu, bias=b1[:, 0:1],
                                 scale=s1[:, 0:1], alpha=alpha1[:, 0:1])

            ah = [apool.tile([128, SH], F32, tag=f"ah{g}", name=f"ah{g}") for g in range(2)]
            aw = [apool.tile([128, W], F32, tag=f"aw{g}", name=f"aw{g}") for g in range(2)]
            for og in range(2):
                ps_a = pspool2.tile([128, SH], F32, tag="psah")
                nc.tensor.matmul(ps_a[:], chT[:, og * 128:(og + 1) * 128],
                                 y[:, 0:SH], start=True, stop=True)
                nc.scalar.activation(ah[og][:], ps_a[:], AF.Sigmoid)
                ps_w = pspool2.tile([128, W], F32, tag="psaw")
                nc.tensor.matmul(ps_w[:], cwT[:, og * 128:(og + 1) * 128],
                                 y[:, SH:], start=True, stop=True)
                nc.scalar.activation(aw[og][:], ps_w[:], AF.Sigmoid)

            # --- attention apply + final store ---
            for t in range(NS):
                r0 = t * SS
                for og in range(2):
                    op_l = wpool.tile([128, SPIX], F32, tag=f"s{og}", name=f"opl{og}_{t}")
                    nc.gpsimd.dma_start(op_l[:],
                                      outp_dram[og][:, r0 * W:(r0 + SS) * W])
                    o3 = op_l.rearrange("p (h w) -> p h w", w=W)
                    fin = wpool.tile([128, SPIX], F32, tag=f"a{og}", name=f"fin{og}_{t}")
                    f3 = fin.rearrange("p (h w) -> p h w", w=W)
                    for i in range(SS):
                        nc.vector.scalar_tensor_tensor(
                            f3[:, i, :], o3[:, i, :],
                            ah[og][:, r0 + i:r0 + i + 1], aw[og][:],
                            ALU.mult, ALU.mult)
                    nc.sync.dma_start(out_t[og][:, r0 * W:(r0 + SS) * W], fin[:])

    nc.compile()
    return nc


# ---- host sharding -----------------------------------------------------------
def shard_inputs(x, consts):
    x = np.asarray(x, dtype=np.float32)
    in_maps = []
    base = {}
    for g in range(2):
        cs = slice(g * 128, (g + 1) * 128)
        base[f"coefH{g}"] = consts["coefH"][cs]
        base[f"coefW{g}"] = consts["coefW"][cs]
        base[f"anch{g}"] = consts["anch"][cs]
        base[f"wfT{g}"] = consts["wfT"][cs].copy()
        base[f"bias{g}"] = consts["bias"][cs, None].copy()
        base[f"palpha{g}"] = consts["alpha"][cs, None].copy()
        base[f"c1T{g}"] = consts["c1T"][cs].copy()
    base["chT"] = consts["chT"]
    base["cwT"] = consts["cwT"]
    base["s1"] = consts["s1"][:, None].copy()
    base["b1"] = consts["b1"][:, None].copy()
    base["alpha1"] = consts["alpha1"][:, None].copy()

    for core in range(NCORES):
        b, j = divmod(core, 4)
        h0 = j * SH
        m = dict(base)
        for g in range(2):
            xp = np.zeros((128, HIN, WPAD), dtype=np.float32)
            lo, hi = h0 - HALO, h0 + SH + HALO
            slo, shi = max(lo, 0), min(hi, H)
            xp[:, slo - lo:shi - lo, HALO:HALO + W] = \
                x[b, g * 128:(g + 1) * 128, slo:shi, :]
            m[f"x{g}"] = xp.reshape(128, HIN * WPAD)
        in_maps.append(m)
    return in_maps


def prepare(x, params):
    consts = fold_constants(params)
    key = (tuple(consts["offs_h"]), tuple(consts["offs_w"]))
    if key not in _CACHE:
        _CACHE[key] = build_program(consts["offs_h"], consts["offs_w"])
    nc = _CACHE[key]
    in_maps = shard_inputs(x, consts)
    return nc, in_maps


def kernel(x, params):
    global LAST_RESULTS
    nc, in_maps = prepare(x, params)
    trace = os.environ.get("KERNEL_TRACE", "0") == "1"
    res = run_bass_kernel_spmd(nc, in_maps, core_ids=list(range(NCORES)),
                               trace=trace)
    LAST_RESULTS = res

    out = np.empty((B, C, H, W), dtype=np.float32)
    for core in range(NCORES):
        b, j = divmod(core, 4)
        h0 = j * SH
        for g in range(2):
            out[b, g * 128:(g + 1) * 128, h0:h0 + SH, :] = \
                res.results[core][f"out{g}"].reshape(128, SH, W)
    return out


# revision 13
# speedup vs baseline: 6.7535x; 6.7535x over previous
"""Trainium2 Bass kernel for nn_Axial_PFCU_DNAS (dense_cnn).

kernel(**inputs) takes the FULL inputs (x [2,256,224,224] f32 + params),
shards across 8 NeuronCores (2 batches x 4 H-slabs of 56 rows), runs a
Bass/Tile kernel per core, and returns the FULL f32 output.

Per-core dataflow (all data bf16, f32 PSUM accumulation):
  1. Axial stencils on the PE as per-channel Toeplitz matmuls:
     - H direction: stationary = per-channel [88,56] Toeplitz (deform-H taps
       + 2x residual), moving = x in H-transposed layout [h', (c,w)].
     - W direction: stationary = x in W-transposed layout [w', (c,h)],
       moving = per-channel [Kq,56] Toeplitz chunks; accumulates into the
       same [h_out, w] psum as the H matmuls.
  2. Evacuate s-psums to bf16 SBUF (DVE/ACT), flip layout to [c, pix] via
     DRAM staging (strided DMA writes, contiguous read-back).
  3. Pointwise 256x256 conv as matmul over flipped s, with the anchor branch
     (x + 3x1 + 1x3 dwconv, BN-folded) fused as 5 extra diagonal K-blocks
     over a host-supplied [c, pix] copy of x.
  4. PReLU (per-channel alpha) on PSUM evac, with row-wise accum_out
     providing the CoordAtt H-pool for free; W-pool partials all-reduced
     across the 4 slab cores of each batch; tiny CoordAtt convs; attention
     applied per-row with fused scalar_tensor_tensor.

Self-contained: hardcodes shapes; only needs the container's /opt/trn_rl_repo.
"""

import math
import os
import sys

sys.path.insert(0, "/opt/trn_rl_repo")

import numpy as np

import concourse.bass as bass
import concourse.bacc as bacc
import concourse.mybir as mybir
import concourse.tile as tile
from concourse.bass_utils import run_bass_kernel_spmd

F32 = mybir.dt.float32
BF16 = mybir.dt.bfloat16
AF = mybir.ActivationFunctionType
ALU = mybir.AluOpType

# ---- problem constants -------------------------------------------------------
B, C, H, W = 2, 256, 224, 224
MIP = 8
EPS = 1e-5
NCORES = 8
SH = 56                  # output rows per core
HALO = 16                # max |offset| of any deform tap
HIN = SH + 2 * HALO      # 88 rows in the H-transposed slab
WP = W + 2               # 226: 1-col zero pad per channel row
PIX = SH * W             # 12544
NQ = 4                   # w_out chunks of 56
WQ = SH                  # 56 w_out per chunk
# w' windows per chunk (global start, size); chunk q covers w_out [56q,56q+56)
WWIN = [(0, 72), (40, 88), (96, 88), (152, 72)]
CCH = 16                 # channels per streaming chunk in the stencil phase
RPC = 2                  # rows per pw chunk
PWCH = RPC * W           # 448 pw chunk width
NPW = SH // RPC          # 28 pw chunks
XFROWS = SH + 2          # xf has 1 halo row each side

ANCH_TAPS = [(0, 0), (-1, 0), (1, 0), (0, -1), (0, 1)]

LAST_RESULTS = None


# ---- host-side constant folding ---------------------------------------------
def _np(v):
    return np.asarray(v, dtype=np.float32)


def _bn_fold(p):
    s = _np(p["gamma"]) / np.sqrt(_np(p["var"]) + EPS)
    b = _np(p["beta"]) - _np(p["mean"]) * s
    return s, b


def _tap_coefs(r, w):
    r = max(float(r), 1.0)
    d = {}
    for i in range(5):
        s = (i - 2) * r
        i0 = math.floor(s)
        f = s - i0
        for dd, wt in ((i0, 1.0 - f), (i0 + 1, f)):
            if wt != 0.0:
                d[dd] = d.get(dd, 0.0) + wt * _np(w)[:, i]
    return d


def _merge(a, b):
    out = dict(a)
    for k, v in b.items():
        out[k] = out.get(k, 0.0) + v
    return out


def fold_constants(params):
    p = params
    cH = _merge(_tap_coefs(p["r_m"], p["wm_h"]), _tap_coefs(p["r_l"], p["wl_h"]))
    cW = _merge(_tap_coefs(p["r_m"], p["wm_w"]), _tap_coefs(p["r_l"], p["wl_w"]))
    cH[0] = cH.get(0, 0.0) + 2.0  # the two residual x terms of (m + l)
    cH = {d: np.broadcast_to(v, (C,)).astype(np.float32) for d, v in cH.items()}
    cW = {d: np.broadcast_to(v, (C,)).astype(np.float32) for d, v in cW.items()}

    sf, bf = _bn_fold(p["bn_fuse"])
    wf = _np(p["pw_fuse_w"])
    wfT = (wf * sf[:, None]).T.copy()  # [C, O]

    sa, sb = _bn_fold(p["dg_bn"])
    wh = _np(p["dg_wh"])[:, 0, :, 0]
    ww = _np(p["dg_ww"])[:, 0, 0, :]
    anch = np.stack(
        [
            sa * (1.0 + wh[:, 1] + ww[:, 1]),
            sa * wh[:, 0],
            sa * wh[:, 2],
            sa * ww[:, 0],
            sa * ww[:, 2],
        ],
        1,
    ).astype(np.float32)  # [C, 5] in ANCH_TAPS order

    bias = (bf + sb).astype(np.float32)
    alpha = _np(p["act_alpha"])

    ca = p["ca"]
    s1, b1 = _bn_fold(ca["bn1"])
    c1T = (_np(ca["conv1_w"]) / 224.0).T.copy()
    chT = _np(ca["convh_w"]).T.copy()
    cwT = _np(ca["convw_w"]).T.copy()
    alpha1 = _np(ca["act_alpha"])

    return dict(
        cH=cH, cW=cW, wfT=wfT, anch=anch, bias=bias, alpha=alpha,
        c1T=c1T, chT=chT, cwT=cwT, s1=s1, b1=b1, alpha1=alpha1,
    )


def build_toeplitz(consts):
    """Per-channel stencil matrices, channel-major packed."""
    cH, cW = consts["cH"], consts["cW"]
    thm = np.zeros((HIN, C, SH), np.float32)
    for d, coef in cH.items():
        for m in range(SH):
            r = m + d + HALO
            if 0 <= r < HIN:
                thm[r, :, m] = coef
    tws = []
    for q in range(NQ):
        wlo, k = WWIN[q]
        t = np.zeros((k, C, WQ), np.float32)
        for d, coef in cW.items():
            for n in range(WQ):
                r = 56 * q + n + d - wlo
                if 0 <= r < k and 0 <= 56 * q + n + d < W:
                    t[r, :, n] = coef
        tws.append(t.reshape(k, C * WQ))
    return thm.reshape(HIN, C * SH), tws


# ---- bass program ------------------------------------------------------------
_CACHE = {}


def build_program():
    nc = bacc.Bacc("TRN2", target_bir_lowering=False, debug=False,
                   num_devices=NCORES)

    def inp(name, shape, dt=BF16):
        return nc.dram_tensor(name, list(shape), dt, kind="ExternalInput").ap()

    xth_t = inp("xth", (HIN, C * WP))
    xtw_t = [inp(f"xtw{q}", (WWIN[q][1], C * SH)) for q in range(NQ)]
    thm_t = inp("thm", (HIN, C * SH))
    tw_t = [inp(f"tw{q}", (WWIN[q][1], C * WQ)) for q in range(NQ)]
    xf_t = [inp(f"xf{g}", (128, XFROWS * WP)) for g in range(2)]
    dg_t = [[inp(f"dg{t}_{g}", (128, 128)) for g in range(2)] for t in range(5)]
    wfT_t = [inp(f"wfT{g}", (128, 256)) for g in range(2)]
    bias_t = [inp(f"bias{g}", (128, 1), F32) for g in range(2)]
    alpha_t = [inp(f"palpha{g}", (128, 1), F32) for g in range(2)]
    c1T_t = [inp(f"c1T{g}", (128, MIP), F32) for g in range(2)]
    chT_t = inp("chT", (MIP, 256), F32)
    cwT_t = inp("cwT", (MIP, 256), F32)
    s1_t = inp("s1", (MIP, 1), F32)
    b1_t = inp("b1", (MIP, 1), F32)
    alpha1_t = inp("alpha1", (MIP, 1), F32)

    out_t = [
        nc.dram_tensor(f"out{g}", [128, PIX], F32, kind="ExternalOutput").ap()
        for g in range(2)
    ]

    NCH = C // CCH  # 8 streaming chunks

    with tile.TileContext(nc) as tc:
        with (
            tc.tile_pool(name="consts", bufs=1) as cpool,
            tc.tile_pool(name="sf", bufs=1) as sfpool,
            tc.tile_pool(name="outp", bufs=1) as opool,
            tc.tile_pool(name="acc", bufs=1) as apool,
            tc.tile_pool(name="dram", bufs=1, space="DRAM") as dpool,
            tc.tile_pool(name="pst", bufs=4, space="PSUM") as pst,
            tc.tile_pool(name="psw", bufs=2, space="PSUM") as psw,
            tc.tile_pool(name="psc", bufs=1, space="PSUM") as psc,
        ):
            # --- small constants ---
            def cload(ap, shape, tag, dt=F32):
                t = cpool.tile(list(shape), dt, tag=tag, name=tag)
                nc.sync.dma_start(t[:], ap)
                return t

            wfT = [cload(wfT_t[g], (128, 256), f"wf{g}", BF16) for g in range(2)]
            dg = [[cload(dg_t[t][g], (128, 128), f"dg{t}_{g}", BF16)
                   for g in range(2)] for t in range(5)]
            bias = [cload(bias_t[g], (128, 1), f"bi{g}") for g in range(2)]
            alpha = [cload(alpha_t[g], (128, 1), f"al{g}") for g in range(2)]
            c1T = [cload(c1T_t[g], (128, MIP), f"c1{g}") for g in range(2)]
            chT = cload(chT_t, (MIP, 256), "chT")
            cwT = cload(cwT_t, (MIP, 256), "cwT")
            s1 = cload(s1_t, (MIP, 1), "s1")
            b1 = cload(b1_t, (MIP, 1), "b1")
            alpha1 = cload(alpha1_t, (MIP, 1), "alpha1")

            xh = [apool.tile([128, SH], F32, tag=f"xh{g}", name=f"xh{g}")
                  for g in range(2)]
            xw = [apool.tile([128, W], F32, tag=f"xw{g}", name=f"xw{g}")
                  for g in range(2)]
            sD = dpool.tile([C, PIX], BF16, name="sD")
            sDv = sD.rearrange("c (h w) -> h c w", w=W)  # (h, c, w) strided view

            # ---- Phase A: stencils, streamed by 32-channel chunks ----
            with (
                tc.tile_pool(name="xs", bufs=2) as xspool,
                tc.tile_pool(name="ev", bufs=4) as evpool,
            ):
                for cc in range(NCH):
                    c0 = cc * CCH
                    xthc = xspool.tile([HIN, CCH * WP], BF16, tag="xth",
                                       name=f"xth_{cc}")
                    nc.sync.dma_start(xthc[:], xth_t[:, c0 * WP:(c0 + CCH) * WP])
                    thmc = xspool.tile([HIN, CCH * SH], BF16, tag="thm",
                                       name=f"thm_{cc}")
                    nc.sync.dma_start(thmc[:], thm_t[:, c0 * SH:(c0 + CCH) * SH])
                    xtwc = []
                    twc = []
                    for q in range(NQ):
                        k = WWIN[q][1]
                        tt = xspool.tile([k, CCH * SH], BF16, tag=f"xtw{q}",
                                         name=f"xtw{q}_{cc}")
                        nc.sync.dma_start(tt[:], xtw_t[q][:, c0 * SH:(c0 + CCH) * SH])
                        xtwc.append(tt)
                        tt2 = xspool.tile([k, CCH * WQ], BF16, tag=f"tw{q}",
                                          name=f"tw{q}_{cc}")
                        nc.sync.dma_start(tt2[:], tw_t[q][:, c0 * WQ:(c0 + CCH) * WQ])
                        twc.append(tt2)

                    for qt in range(CCH // 4):  # 4 channels per psum tile
                        ps = pst.tile([128, 448], F32, tag="st", name=f"st_{cc}_{qt}")
                        for j in range(4):
                            lc = qt * 4 + j      # channel within chunk
                            a, b = j >> 1, j & 1
                            dst = ps[a * 64:a * 64 + 56, b * 224:(b + 1) * 224]
                            nc.tensor.matmul(
                                dst,
                                thmc[:, lc * SH:(lc + 1) * SH],
                                xthc[:, lc * WP + 1:lc * WP + 1 + W],
                                start=True, stop=False)
                            for q in range(NQ):
                                nc.tensor.matmul(
                                    dst[:, q * WQ:(q + 1) * WQ],
                                    xtwc[q][:, lc * SH:(lc + 1) * SH],
                                    twc[q][:, lc * WQ:(lc + 1) * WQ],
                                    start=False, stop=(q == NQ - 1))
                        ev = evpool.tile([128, 448], BF16, tag="ev",
                                         name=f"ev_{cc}_{qt}")
                        if qt % 2 == 0:
                            nc.vector.tensor_copy(ev[:], ps[:])
                        else:
                            nc.scalar.copy(ev[:], ps[:])
                        # flip-write: one DMA per row-half; dst (h, b, w) strided
                        gc0 = c0 + qt * 4
                        for a in range(2):
                            nc.sync.dma_start(
                                sDv[:, gc0 + 2 * a:gc0 + 2 * a + 2, :],
                                ev[a * 64:a * 64 + 56, :]
                                .rearrange("h (b w) -> h b w", w=W))

            # ---- Phase B: flip read-back ----
            sF = [sfpool.tile([128, PIX], BF16, tag=f"sF{g}", name=f"sF{g}")
                  for g in range(2)]
            for g in range(2):
                nc.sync.dma_start(sF[g][:], sD[g * 128:(g + 1) * 128, :])

            # ---- Phase C: pw conv + anchor K-ext + PReLU + pools ----
            outp = [opool.tile([128, PIX], BF16, tag=f"outp{g}", name=f"op{g}")
                    for g in range(2)]
            with tc.tile_pool(name="xfp", bufs=1) as xfpool:
                xf = []
                for g in range(2):
                    t = xfpool.tile([128, XFROWS * WP], BF16, tag=f"xf{g}",
                                    name=f"xf{g}")
                    nc.sync.dma_start(t[:], xf_t[g])
                    xf.append(t)

                for og in range(2):
                    x3 = xf[og].rearrange("p (h w) -> p h w", w=WP)
                    for ck in range(NPW):
                        r0 = ck * RPC
                        ps = psw.tile([128, PWCH], F32, tag="pw",
                                      name=f"pw_{og}_{ck}")
                        for kg in range(2):
                            nc.tensor.matmul(
                                ps[:], wfT[kg][:, og * 128:(og + 1) * 128],
                                sF[kg][:, r0 * W:(r0 + RPC) * W],
                                start=(kg == 0), stop=False)
                        for t, (dh, dw) in enumerate(ANCH_TAPS):
                            nc.tensor.matmul(
                                ps[:], dg[t][og][:],
                                x3[:, 1 + r0 + dh:1 + r0 + dh + RPC,
                                   1 + dw:1 + dw + W],
                                start=False, stop=(t == 4))
                        for r in range(RPC):
                            nc.scalar.activation(
                                outp[og][:, (r0 + r) * W:(r0 + r + 1) * W],
                                ps[:, r * W:(r + 1) * W],
                                AF.Prelu, bias=bias[og][:, 0:1], scale=1.0,
                                alpha=alpha[og][:, 0:1],
                                accum_out=xh[og][:, r0 + r:r0 + r + 1])
                    o3t = outp[og].rearrange("p (h w) -> p w h", w=W)
                    nc.vector.tensor_reduce(xw[og][:], o3t,
                                            axis=mybir.AxisListType.X,
                                            op=ALU.add)

            # ---- Phase D: all-reduce W-pool + CoordAtt bottleneck ----
            cc_in = dpool.tile([C, W], F32, name="ccin")
            cc_out = dpool.tile([C, W], F32, name="ccout")
            for og in range(2):
                nc.sync.dma_start(cc_in[og * 128:(og + 1) * 128, :], xw[og][:])
            nc.gpsimd.collective_compute(
                "AllReduce", ALU.add,
                replica_groups=[[0, 1, 2, 3], [4, 5, 6, 7]],
                ins=[cc_in.opt()], outs=[cc_out.opt()],
            )
            xwsum = [apool.tile([128, W], F32, tag=f"xws{g}", name=f"xws{g}")
                     for g in range(2)]
            for og in range(2):
                nc.sync.dma_start(xwsum[og][:], cc_out[og * 128:(og + 1) * 128, :])

            ps_y = psc.tile([MIP, SH + W], F32, tag="ca", name="psy")
            nc.tensor.matmul(ps_y[:, 0:SH], c1T[0][:], xh[0][:],
                             start=True, stop=False)
            nc.tensor.matmul(ps_y[:, 0:SH], c1T[1][:], xh[1][:],
                             start=False, stop=True)
            nc.tensor.matmul(ps_y[:, SH:], c1T[0][:], xwsum[0][:],
                             start=True, stop=False)
            nc.tensor.matmul(ps_y[:, SH:], c1T[1][:], xwsum[1][:],
                             start=False, stop=True)
            y = apool.tile([MIP, SH + W], F32, tag="y", name="y")
            nc.scalar.activation(y[:], ps_y[:], AF.Prelu, bias=b1[:, 0:1],
                                 scale=s1[:, 0:1], alpha=alpha1[:, 0:1])

            ah = [apool.tile([128, SH], F32, tag=f"ah{g}", name=f"ah{g}")
                  for g in range(2)]
            aw = [apool.tile([128, W], F32, tag=f"aw{g}", name=f"aw{g}")
                  for g in range(2)]
            for og in range(2):
                ps_a = psc.tile([128, SH], F32, tag="ca", name=f"psah{og}")
                nc.tensor.matmul(ps_a[:], chT[:, og * 128:(og + 1) * 128],
                                 y[:, 0:SH], start=True, stop=True)
                nc.scalar.activation(ah[og][:], ps_a[:], AF.Sigmoid)
                ps_w = psc.tile([128, W], F32, tag="ca", name=f"psaw{og}")
                nc.tensor.matmul(ps_w[:], cwT[:, og * 128:(og + 1) * 128],
                                 y[:, SH:], start=True, stop=True)
                nc.scalar.activation(aw[og][:], ps_w[:], AF.Sigmoid)

            # ---- Phase E: attention apply + store ----
            with tc.tile_pool(name="fin", bufs=4) as fpool:
                for og in range(2):
                    o3 = outp[og].rearrange("p (h w) -> p h w", w=W)
                    for ck in range(NPW):
                        r0 = ck * RPC
                        fin = fpool.tile([128, PWCH], F32, tag="fin",
                                         name=f"fin_{og}_{ck}")
                        f3 = fin.rearrange("p (h w) -> p h w", w=W)
                        for r in range(RPC):
                            nc.vector.scalar_tensor_tensor(
                                f3[:, r, :], o3[:, r0 + r, :],
                                ah[og][:, r0 + r:r0 + r + 1], aw[og][:],
                                ALU.mult, ALU.mult)
                        nc.sync.dma_start(
                            out_t[og][:, r0 * W:(r0 + RPC) * W], fin[:])

    nc.compile()
    return nc


# ---- host sharding -----------------------------------------------------------
def shard_inputs(x, consts):
    import ml_dtypes
    bf16 = ml_dtypes.bfloat16
    x = np.asarray(x, dtype=np.float32)
    thm, tws = build_toeplitz(consts)

    base = {
        "thm": np.ascontiguousarray(thm.astype(bf16)),
        "chT": consts["chT"], "cwT": consts["cwT"],
        "s1": consts["s1"][:, None].copy(), "b1": consts["b1"][:, None].copy(),
        "alpha1": consts["alpha1"][:, None].copy(),
    }
    for q in range(NQ):
        base[f"tw{q}"] = np.ascontiguousarray(tws[q].astype(bf16))
    for g in range(2):
        cs = slice(g * 128, (g + 1) * 128)
        base[f"wfT{g}"] = np.ascontiguousarray(consts["wfT"][cs].astype(bf16))
        base[f"bias{g}"] = consts["bias"][cs, None].copy()
        base[f"palpha{g}"] = consts["alpha"][cs, None].copy()
        base[f"c1T{g}"] = consts["c1T"][cs].copy()
        for t in range(5):
            d = np.zeros((128, 128), np.float32)
            np.fill_diagonal(d, consts["anch"][cs, t])
            base[f"dg{t}_{g}"] = d.astype(bf16)

    xb = x.astype(bf16)
    in_maps = []
    for core in range(NCORES):
        b, j = divmod(core, 4)
        h0 = j * SH
        m = dict(base)

        # xth [HIN, (c, wp)]
        xth = np.zeros((HIN, C, WP), bf16)
        lo, hi = h0 - HALO, h0 + SH + HALO
        slo, shi = max(lo, 0), min(hi, H)
        xth[slo - lo:shi - lo, :, 1:1 + W] = \
            xb[b, :, slo:shi, :].transpose(1, 0, 2)
        m["xth"] = np.ascontiguousarray(xth.reshape(HIN, C * WP))

        # xtw window tiles [Kq, (c, h)]
        slab = xb[b, :, h0:h0 + SH, :]  # [C, SH, W]
        wt = slab.transpose(2, 0, 1)    # [W, C, SH]
        for q in range(NQ):
            wlo, k = WWIN[q]
            m[f"xtw{q}"] = np.ascontiguousarray(
                wt[wlo:wlo + k].reshape(k, C * SH))

        # xf [2][128, (rows 58, wp)]
        for g in range(2):
            xfv = np.zeros((128, XFROWS, WP), bf16)
            lo2, hi2 = h0 - 1, h0 + SH + 1
            s2lo, s2hi = max(lo2, 0), min(hi2, H)
            xfv[:, s2lo - lo2:s2hi - lo2, 1:1 + W] = \
                xb[b, g * 128:(g + 1) * 128, s2lo:s2hi, :]
            m[f"xf{g}"] = np.ascontiguousarray(xfv.reshape(128, XFROWS * WP))

        in_maps.append(m)
    return in_maps


def prepare(x, params):
    consts = fold_constants(params)
    if "nc" not in _CACHE:
        _CACHE["nc"] = build_program()
    nc = _CACHE["nc"]
    in_maps = shard_inputs(x, consts)
    return nc, in_maps


def kernel(x, params):
    global LAST_RESULTS
    nc, in_maps = prepare(x, params)
    trace = os.environ.get("KERNEL_TRACE", "0") == "1"
    res = run_bass_kernel_spmd(nc, in_maps, core_ids=list(range(NCORES)),
                               trace=trace)
    LAST_RESULTS = res

    out = np.empty((B, C, H, W), dtype=np.float32)
    for core in range(NCORES):
        b, j = divmod(core, 4)
        h0 = j * SH
        for g in range(2):
            out[b, g * 128:(g + 1) * 128, h0:h0 + SH, :] = \
                res.results[core][f"out{g}"].reshape(128, SH, W)
    return out
